# revision 1
# baseline (speedup 1.0000x reference)
"""ExtendedMoCHILoss on 8 Trainium2 NeuronCores (Bass/Tile).

Strategy (memory-bound problem, 144MiB of rows to stream):
  - Shard hard_negatives rows (65536 -> 8192/core) and positives rows
    (8192 -> 1024/core) across the 8 cores.
  - Per row we only ever need dot(row, anchor_raw) and sumsq(row):
        logit = dot * rsqrt(sumsq_row) * rsqrt(sumsq_anchor) / tau
    so rows are never normalized/materialized.
  - The 128 synthesized negatives depend on 192 indexed rows of h; the host
    gathers those rows and every core receives them (same program on all
    cores), but only core 0 counts their exp-sum via a mask input.
    Closed form used on-device (h_n = row/||row||, av = anchor/||anchor||):
      hardest: u = (1-a)*h_n[m] + a*av
        u.av   = c + a*(1-c)          where c = cos(h[m], anchor)
        |u|^2  = 1 - 2a(1-a)(1-c)
      harder:  v = b*h_n[x] + (1-b)*h_n[y]
        v.av   = cy + b*(cx-cy)
        |v|^2  = 1 - 2b(1-b)(1-cxy),  cxy = cos(h[x], h[y])
  - exp-sums are combined across cores with an on-device AllGather; the
    positive-term mean needs the global sum S, then a second AllGather
    combines the per-core partial loss sums.  Every core computes the same
    final loss; the host reads core 0's scalar.

Engine split per [128,512] f32 tile (DMA ~51us/core is the roofline):
  ACT          : Square + fused row-accumulate -> sumsq column
                 (ACT stays on ONE function; table reloads cost ~1.3us)
  DVE / GPSIMD : tensor_mul with broadcast anchor (alternating tiles)
  DVE          : row-reduce of the product via tensor_scalar*1.0 with
                 accum_out (2x fp32 mode, ~2x faster than tensor_reduce)
  (NOTE: vector.tensor_tensor_reduce would fuse mul+reduce but crashes the
   exec unit with this toolchain - verified empirically - so it is avoided.)
"""

import os
import sys

sys.path.insert(0, "/opt/trn_rl_repo")

import numpy as np

import concourse.bass as bass
import concourse.bacc as bacc
import concourse.tile as tile
from concourse import mybir
from concourse.bass_utils import run_bass_kernel_spmd

N_CORES = 8
D = 512
N_POS = 8192
N_HARD = 65536
N_MIX = 64
HS = N_HARD // N_CORES  # 8192 h rows per core
PS = N_POS // N_CORES  # 1024 p rows per core
P = 128
HT = HS // P  # 64 h tiles per core
PT = PS // P  # 8 p tiles per core
INV_TAU = 10.0
EPS_DENOM = 1e-8
EPS_NSQ = 1e-24  # max(sqrt(q),1e-12) == sqrt(max(q,1e-24)) for q>=0

F32 = mybir.dt.float32
ActF = mybir.ActivationFunctionType
Alu = mybir.AluOpType
AXX = mybir.AxisListType.X

_CACHED_NC = None


def _bcast_ap(ap, parts):
    """Partition-broadcast read of a single-partition DRAM AP."""
    return bass.AP(tensor=ap.tensor, offset=ap.offset, ap=[[0, parts], ap.ap[1]])


def _build(loops=1):
    nc = bacc.Bacc("TRN2", target_bir_lowering=False, debug=False, num_devices=N_CORES)

    hs = nc.dram_tensor("hs", [D, HS], F32, kind="ExternalInput").ap()  # transposed
    ps = nc.dram_tensor("ps", [D, PS], F32, kind="ExternalInput").ap()  # transposed
    anc = nc.dram_tensor("anc", [1, D], F32, kind="ExternalInput").ap()
    gmix = nc.dram_tensor("gmix", [N_MIX, D], F32, kind="ExternalInput").ap()
    gxa = nc.dram_tensor("gxa", [N_MIX, D], F32, kind="ExternalInput").ap()
    gxb = nc.dram_tensor("gxb", [N_MIX, D], F32, kind="ExternalInput").ap()
    araw = nc.dram_tensor("araw", [N_MIX, 1], F32, kind="ExternalInput").ap()
    braw = nc.dram_tensor("braw", [N_MIX, 1], F32, kind="ExternalInput").ap()
    mask = nc.dram_tensor("mask", [1, 1], F32, kind="ExternalInput").ap()
    loss = nc.dram_tensor("loss", [1, 1], F32, kind="ExternalOutput").ap()

    with tile.TileContext(nc) as tc:
        with (
            tc.tile_pool(name="stream", bufs=5) as stream,  # h/p input tiles
            tc.tile_pool(name="sqscr", bufs=4) as sqscr,  # ACT square outputs
            tc.tile_pool(name="prod", bufs=8) as prod_pool,  # mul outputs
            tc.tile_pool(name="tsscr", bufs=8) as tsscr,  # ts-reduce outputs
            tc.tile_pool(name="single", bufs=1) as single,  # persistent small
            tc.tile_pool(name="psum", bufs=4, space="PSUM") as psum,
            tc.tile_pool(name="dram", bufs=4, space="DRAM") as dram,
        ):

            def ts_rowsum(dst_col, src, scr_tag="tss"):
                """dst_col[P,1] = rowsum(src) via DVE tensor_scalar*1.0."""
                scr = tsscr.tile(list(src.shape), F32, tag=scr_tag)
                nc.vector.tensor_scalar(
                    out=scr,
                    in0=src,
                    scalar1=1.0,
                    scalar2=None,
                    op0=Alu.mult,
                    op1=Alu.add,
                    accum_out=dst_col,
                )

            # ---------- setup ----------
            ab = single.tile([P, D], F32, tag="ab")  # raw anchor broadcast
            nc.sync.dma_start(out=ab, in_=_bcast_ap(anc, P))
            mask_col = single.tile([P, 1], F32, tag="maskc")
            nc.sync.dma_start(out=mask_col, in_=_bcast_ap(mask, P))

            # sumsq(anchor) replicated on every partition; inv_na = rsqrt;
            # s_col = inv_na / tau
            aa_scr = sqscr.tile([P, D], F32, tag="sq")
            aa = single.tile([P, 1], F32, tag="aa")
            nc.scalar.activation(out=aa_scr, in_=ab, func=ActF.Square, accum_out=aa)
            nc.vector.tensor_scalar_max(out=aa, in0=aa, scalar1=EPS_NSQ)
            na = single.tile([P, 1], F32, tag="na")
            nc.scalar.sqrt(out=na, in_=aa)
            inv_na = single.tile([P, 1], F32, tag="invna")
            nc.vector.reciprocal(out=inv_na, in_=na)
            s_col = single.tile([P, 1], F32, tag="scol")
            nc.vector.tensor_scalar_mul(out=s_col, in0=inv_na, scalar1=INV_TAU)

            ones = single.tile([P, 1], F32, tag="ones")
            nc.vector.memset(ones, 1.0)

            # ---------- synthesized negatives (all cores; masked later) ----
            gtiles = {}
            for name, src in (("A", gmix), ("B", gxa), ("C", gxb)):
                gt = single.tile([N_MIX, D], F32, tag=f"g{name}")
                nc.sync.dma_start(out=gt, in_=src)
                gtiles[name] = gt

            gss = {}
            gdot = {}
            for name, gt in gtiles.items():
                scr = sqscr.tile([N_MIX, D], F32, tag="sq64")
                ss = single.tile([N_MIX, 1], F32, tag=f"ss{name}")
                nc.scalar.activation(out=scr, in_=gt, func=ActF.Square, accum_out=ss)
                gss[name] = ss
                pr = prod_pool.tile([N_MIX, D], F32, tag="prod64")
                nc.vector.tensor_mul(out=pr, in0=gt, in1=ab[0:N_MIX, :])
                dt_ = single.tile([N_MIX, 1], F32, tag=f"dot{name}")
                ts_rowsum(dt_, pr, "tss64")
                gdot[name] = dt_
            prBC = prod_pool.tile([N_MIX, D], F32, tag="prod64")
            nc.vector.tensor_mul(out=prBC, in0=gtiles["B"], in1=gtiles["C"])
            dBC = single.tile([N_MIX, 1], F32, tag="dotBC")
            ts_rowsum(dBC, prBC, "tss64")

            # cosines with the anchor: c = dot * rsqrt(ssq) * inv_na
            ginv = {}
            for name in ("A", "B", "C"):
                t = single.tile([N_MIX, 1], F32, tag=f"ginv{name}")
                nc.vector.tensor_scalar_max(out=t, in0=gss[name], scalar1=EPS_NSQ)
                nc.scalar.sqrt(out=t, in_=t)
                nc.vector.reciprocal(out=t, in_=t)
                ginv[name] = t
            gcos = {}
            for name in ("A", "B", "C"):
                c = single.tile([N_MIX, 1], F32, tag=f"gcos{name}")
                nc.vector.tensor_mul(out=c, in0=gdot[name], in1=ginv[name])
                nc.vector.tensor_mul(out=c, in0=c, in1=inv_na[0:N_MIX, :])
                gcos[name] = c
            cBC = single.tile([N_MIX, 1], F32, tag="cosBC")
            nc.vector.tensor_mul(out=cBC, in0=dBC, in1=ginv["B"])
            nc.vector.tensor_mul(out=cBC, in0=cBC, in1=ginv["C"])

            spre = single.tile([N_MIX, 2], F32, tag="spre")

            def _mix_logit_pre(out_ap, coef, cdot, cmix, tagp):
                # out = cdot * rsqrt(1 - 2*coef*(1-coef)*(1-cmix))
                w = single.tile([N_MIX, 1], F32, tag=f"w{tagp}")
                nc.vector.tensor_scalar(
                    out=w, in0=coef, scalar1=-1.0, scalar2=1.0,
                    op0=Alu.mult, op1=Alu.add,
                )
                nc.vector.tensor_mul(out=w, in0=w, in1=coef)  # coef*(1-coef)
                omc = single.tile([N_MIX, 1], F32, tag=f"omc{tagp}")
                nc.vector.tensor_scalar(
                    out=omc, in0=cmix, scalar1=-1.0, scalar2=1.0,
                    op0=Alu.mult, op1=Alu.add,
                )
                nsq = single.tile([N_MIX, 1], F32, tag=f"nsq{tagp}")
                nc.vector.tensor_mul(out=nsq, in0=w, in1=omc)
                nc.vector.tensor_scalar(
                    out=nsq, in0=nsq, scalar1=-2.0, scalar2=1.0,
                    op0=Alu.mult, op1=Alu.add,
                )
                nc.vector.tensor_scalar_max(out=nsq, in0=nsq, scalar1=EPS_NSQ)
                nc.scalar.sqrt(out=nsq, in_=nsq)
                nc.vector.reciprocal(out=nsq, in_=nsq)
                nc.vector.tensor_mul(out=out_ap, in0=cdot, in1=nsq)

            # hardest: alpha = araw*0.4+0.1 ; u.av = cA + alpha*(1-cA)
            al = single.tile([N_MIX, 1], F32, tag="al")
            nc.sync.dma_start(out=al, in_=araw)
            nc.vector.tensor_scalar(
                out=al, in0=al, scalar1=0.4, scalar2=0.1, op0=Alu.mult, op1=Alu.add
            )
            udot = single.tile([N_MIX, 1], F32, tag="udot")
            nc.vector.tensor_scalar(
                out=udot, in0=gcos["A"], scalar1=-1.0, scalar2=1.0,
                op0=Alu.mult, op1=Alu.add,
            )
            nc.vector.tensor_mul(out=udot, in0=udot, in1=al)
            nc.vector.tensor_add(out=udot, in0=udot, in1=gcos["A"])
            _mix_logit_pre(spre[:, 0:1], al, udot, gcos["A"], "u")

            # harder: beta = braw*0.4+0.3 ; v.av = cC + beta*(cB-cC)
            be = single.tile([N_MIX, 1], F32, tag="be")
            nc.sync.dma_start(out=be, in_=braw)
            nc.vector.tensor_scalar(
                out=be, in0=be, scalar1=0.4, scalar2=0.3, op0=Alu.mult, op1=Alu.add
            )
            vdot = single.tile([N_MIX, 1], F32, tag="vdot")
            nc.vector.tensor_sub(out=vdot, in0=gcos["B"], in1=gcos["C"])
            nc.vector.tensor_mul(out=vdot, in0=vdot, in1=be)
            nc.vector.tensor_add(out=vdot, in0=vdot, in1=gcos["C"])
            _mix_logit_pre(spre[:, 1:2], be, vdot, cBC, "v")

            sexp_scr = sqscr.tile([N_MIX, 2], F32, tag="sexpscr")
            ssum = single.tile([N_MIX, 1], F32, tag="ssum")
            nc.scalar.activation(
                out=sexp_scr, in_=spre, func=ActF.Exp, scale=INV_TAU, accum_out=ssum
            )
            msynth = single.tile([N_MIX, 1], F32, tag="msynth")
            nc.vector.tensor_scalar_mul(
                out=msynth, in0=ssum, scalar1=mask_col[0:N_MIX, :]
            )

            # ---------- main streams (PE-based, transposed layout) ----------
            # Inputs arrive host-transposed: hs=[D, HS], ps=[D, PS].  A 2MiB
            # DMA brings in one d-chunk x 4096 rows ([128, 4096], 16KiB
            # contiguous per partition).  Per 512-row group:
            #   dot  row: psum[0,:] += avT_c^T @ x        (fp32r, full rate)
            #   ssq  row: psum[1,:] += ones^T @ square(x) (fp32r)
            # accumulated over the 4 d-chunks; square(x) is the only
            # vector-engine pass (split ACT/DVE/GPSIMD).  PSUM [2,512] groups
            # are evicted by DVE into [2, N] strips; a DRAM bounce reshapes
            # strips into [128, ncols] column buffers for the tail.
            F32R = mybir.dt.float32r

            # avT: partition p of column c holds anchor[0, c*128+p]
            avt = single.tile([P, 4], F32, tag="avt")
            avt_src = bass.AP(
                tensor=anc.tensor, offset=anc.offset, ap=[[1, P], [P, 4]]
            )
            nc.sync.dma_start(out=avt, in_=avt_src)
            avt_r = single.tile([P, 4], F32R, tag="avtr")
            nc.vector.tensor_copy(out=avt_r, in_=avt)
            ones_r = single.tile([P, 1], F32R, tag="onesr")
            nc.vector.tensor_copy(out=ones_r, in_=ones)

            hdstrip = single.tile([1, HS], F32, tag="hdstrip")
            hqstrip = single.tile([1, HS], F32, tag="hqstrip")
            pdstrip = single.tile([1, PS], F32, tag="pdstrip")
            pqstrip = single.tile([1, PS], F32, tag="pqstrip")

            def stream_pe(src_t, cstrip, qstrip, rs_groups, sl=[0]):
                # src_t: [D, nrows] DRAM (transposed rows).  Per 512-row group
                # two PSUM rows accumulated over the 4 d-chunks:
                #   combo: Sum_d (x+av)^2 = ssq + 2*dot + ssq_a   (ACT Square
                #          with per-partition bias=avT, fp32r out)
                #   ssq  : Sum_d x^2                (DVE/GPSIMD mul, fp32r out)
                # Both reduced on PE via ones-matmuls at full fp32r rate; the
                # dot column is recovered at the tail as (combo-ssq-aa)/2.
                for row0, nrows in rs_groups:
                    ngrp = nrows // 512
                    pcs = [
                        psum.tile([1, 512], F32, tag="pc", name=f"pc{row0}_{i}")
                        for i in range(ngrp)
                    ]
                    pqs = [
                        psum.tile([1, 512], F32, tag="pq", name=f"pq{row0}_{i}")
                        for i in range(ngrp)
                    ]
                    for c in range(4):
                        xt = stream.tile([P, nrows], F32, tag="xt")
                        nc.sync.dma_start(
                            out=xt,
                            in_=src_t[c * P : (c + 1) * P, row0 : row0 + nrows],
                        )
                        for gg2 in range(ngrp // 2):
                            # 1024-wide elementwise ops (2 groups per op)
                            # amortize per-op overhead; PE still reduces in
                            # 512-wide PSUM-bank sub-slices.
                            xs = xt[:, gg2 * 1024 : (gg2 + 1) * 1024]
                            i = sl[0]
                            sl[0] += 1
                            combo = sqscr.tile([P, 1024], F32R, tag="combo")
                            nc.scalar.activation(
                                out=combo,
                                in_=xs,
                                func=ActF.Square,
                                bias=avt[:, c : c + 1],
                                scale=1.0,
                            )
                            sq = sqscr.tile([P, 1024], F32R, tag="sq")
                            if i % 8 < 5:
                                nc.vector.tensor_mul(out=sq, in0=xs, in1=xs)
                            else:
                                nc.gpsimd.tensor_mul(out=sq, in0=xs, in1=xs)
                            for hh in range(2):
                                gg = gg2 * 2 + hh
                                sub = slice(hh * 512, (hh + 1) * 512)
                                nc.tensor.matmul(
                                    pcs[gg],
                                    lhsT=ones_r[:, 0:1],
                                    rhs=combo[:, sub],
                                    start=(c == 0),
                                    stop=(c == 3),
                                )
                                nc.tensor.matmul(
                                    pqs[gg],
                                    lhsT=ones_r[:, 0:1],
                                    rhs=sq[:, sub],
                                    start=(c == 0),
                                    stop=(c == 3),
                                )
                    for gg in range(ngrp):
                        seg = slice(row0 + gg * 512, row0 + (gg + 1) * 512)
                        nc.vector.tensor_copy(out=cstrip[0:1, seg], in_=pcs[gg])
                        nc.vector.tensor_copy(out=qstrip[0:1, seg], in_=pqs[gg])

            stream_pe(ps, pdstrip, pqstrip, [(0, PS)])
            stream_pe(
                hs, hdstrip, hqstrip,
                [(0, 2048), (2048, 2048), (4096, 2048), (6144, 2048)],
            )

            # strips -> DRAM bounce -> [P, ncols] column buffers
            hb = dram.tile([2, HS], F32, tag="hb")
            pb = dram.tile([2, PS], F32, tag="pb")
            nc.sync.dma_start(out=hb[0:1, :], in_=hdstrip)
            nc.sync.dma_start(out=hb[1:2, :], in_=hqstrip)
            nc.scalar.dma_start(out=pb[0:1, :], in_=pdstrip)
            nc.scalar.dma_start(out=pb[1:2, :], in_=pqstrip)

            def strip_cols(bounce, row, ncols):
                cb = single.tile([P, ncols], F32, tag=f"cb{row}{ncols}")
                src_ap = bass.AP(
                    tensor=bounce.tensor,
                    offset=bounce.offset + row * (ncols * P),
                    ap=[[ncols, P], [1, ncols]],
                )
                nc.sync.dma_start(out=cb, in_=src_ap)
                return cb

            sch = strip_cols(hb, 0, HT)
            ssh = strip_cols(hb, 1, HT)
            scp = strip_cols(pb, 0, PT)
            ssp = strip_cols(pb, 1, PT)

            # dot = (combo - ssq - ssq_anchor) / 2   (aa is per-partition)
            def recover_dot(sc, ss, ncols, tagp):
                dt_ = single.tile([P, ncols], F32, tag=f"dotc{tagp}")
                nc.vector.tensor_sub(out=dt_, in0=sc, in1=ss)
                nc.vector.tensor_scalar(
                    out=dt_, in0=dt_, scalar1=aa, scalar2=0.5,
                    op0=Alu.subtract, op1=Alu.mult,
                )
                return dt_

            doth = recover_dot(sch, ssh, HT, "h")
            dotp = recover_dot(scp, ssp, PT, "p")

            # ---------- logits ----------
            def logits_pre(ss, dot, ncols, tagp):
                # dot * rsqrt(max(ss,eps)), shape [P, ncols]
                inv = single.tile([P, ncols], F32, tag=f"inv{tagp}")
                nc.vector.tensor_scalar_max(out=inv, in0=ss, scalar1=EPS_NSQ)
                nc.scalar.sqrt(out=inv, in_=inv)
                nc.vector.reciprocal(out=inv, in_=inv)
                pre = single.tile([P, ncols], F32, tag=f"pre{tagp}")
                nc.vector.tensor_mul(out=pre, in0=dot, in1=inv)
                return pre

            # ---------- per-core AllGather payload ----------
            # [0:1024]   = this core's positive logits (any order)
            # [1024]     = this core's negatives exp-sum (incl. masked synth)
            # [1025:1032] = zero pad to a 32B-aligned 4128B per-rank buffer
            AGW = 1032
            ag_in = dram.tile([1, AGW], F32, tag="agin")
            ag_out = dram.tile([1, AGW * N_CORES], F32, tag="agout")

            # positive logits: l = (dot * rsqrt(ssq)) * s_col
            pre_p = logits_pre(ssp, dotp, PT, "p")
            lp = single.tile([P, PT], F32, tag="lp")
            nc.vector.tensor_scalar_mul(out=lp, in0=pre_p, scalar1=s_col)
            lp_dst = bass.AP(
                tensor=ag_in.tensor, offset=ag_in.offset, ap=[[PT, P], [1, PT]]
            )
            nc.sync.dma_start(out=lp_dst, in_=lp)

            # negatives: exp(pre * s_col), row-accumulated
            pre_h = logits_pre(ssh, doth, HT, "h")
            hexp_scr = sqscr.tile([P, HT], F32, tag="hexps")
            hsum = single.tile([P, 1], F32, tag="hsum")
            nc.scalar.activation(
                out=hexp_scr, in_=pre_h, func=ActF.Exp, scale=s_col, accum_out=hsum
            )
            # include synthesized negatives (masked; nonzero only on core 0)
            nc.vector.tensor_add(
                out=hsum[0:N_MIX, :], in0=hsum[0:N_MIX, :], in1=msynth
            )
            negp_ps = psum.tile([1, 1], F32, tag="pc", name="negp_ps")
            nc.tensor.matmul(negp_ps, lhsT=hsum, rhs=ones, start=True, stop=True)
            negp = single.tile([1, 8], F32, tag="negp")
            nc.vector.tensor_copy(out=negp[0:1, 0:1], in_=negp_ps)
            nc.vector.memset(negp[0:1, 1:8], 0.0)
            nc.sync.dma_start(out=ag_in[0:1, 1024:AGW], in_=negp)

            nc.gpsimd.collective_compute(
                "AllGather",
                Alu.bypass,
                replica_groups=[list(range(N_CORES))],
                ins=[ag_in.opt()],
                outs=[ag_out.opt()],
            )

            # ---------- finish locally: loss = mean(log1p((S+eps)e^-l)) ----
            # gather all 8192 positive logits -> [P, 64]
            lpa = single.tile([P, N_CORES, PT], F32, tag="lpall")
            lpa_src = bass.AP(
                tensor=ag_out.tensor,
                offset=ag_out.offset,
                ap=[[PT, P], [AGW, N_CORES], [1, PT]],
            )
            nc.sync.dma_start(out=lpa, in_=lpa_src)
            lpa2 = lpa.rearrange("p a b -> p (a b)")
            # S = sum of per-core exp-sums, broadcast on all partitions
            negs = single.tile([P, N_CORES], F32, tag="negs")
            negs_src = bass.AP(
                tensor=ag_out.tensor,
                offset=ag_out.offset + 1024,
                ap=[[0, P], [AGW, N_CORES]],
            )
            nc.sync.dma_start(out=negs, in_=negs_src)
            s_eps = single.tile([P, 1], F32, tag="seps")
            nc.vector.reduce_sum(out=s_eps, in_=negs, axis=AXX)
            nc.vector.tensor_scalar_add(out=s_eps, in0=s_eps, scalar1=EPS_DENOM)

            e = single.tile([P, N_CORES * PT], F32, tag="pe")
            nc.scalar.activation(out=e, in_=lpa2, func=ActF.Exp, scale=-1.0)
            f = single.tile([P, N_CORES * PT], F32, tag="pf")
            nc.vector.tensor_scalar_mul(out=f, in0=e, scalar1=s_eps)
            t = single.tile([P, N_CORES * PT], F32, tag="pt")
            pp = single.tile([P, 1], F32, tag="pp")
            nc.scalar.activation(
                out=t, in_=f, func=ActF.Ln, bias=1.0, scale=1.0, accum_out=pp
            )
            posp_ps = psum.tile([1, 1], F32, tag="pq", name="posp_ps")
            nc.tensor.matmul(posp_ps, lhsT=pp, rhs=ones, start=True, stop=True)
            lsum = single.tile([1, 1], F32, tag="lsum")
            nc.vector.tensor_scalar_mul(
                out=lsum, in0=posp_ps, scalar1=1.0 / N_POS
            )
            nc.sync.dma_start(out=loss, in_=lsum)

    nc.compile()
    return nc


def _get_nc():
    global _CACHED_NC
    if _CACHED_NC is None:
        _CACHED_NC = _build()
    return _CACHED_NC


LAST_RESULTS = None  # BassKernelResults of the most recent run (for profiling)


def _in_maps(anchor, h, p, gm, ga, gb, ar, br):
    maps = []
    for c in range(N_CORES):
        maps.append(
            {
                "hs": np.ascontiguousarray(h[c * HS : (c + 1) * HS].T),
                "ps": np.ascontiguousarray(p[c * PS : (c + 1) * PS].T),
                "anc": anchor,
                "gmix": gm,
                "gxa": ga,
                "gxb": gb,
                "araw": ar,
                "braw": br,
                "mask": np.asarray([[1.0 if c == 0 else 0.0]], dtype=np.float32),
            }
        )
    return maps


def kernel(
    anchor, positives, hard_negatives, mix_idx, idx_a, idx_b, alpha_raw, beta_raw
):
    nc = _get_nc()
    anchor = np.ascontiguousarray(anchor, dtype=np.float32)
    h = np.ascontiguousarray(hard_negatives, dtype=np.float32)
    p = np.ascontiguousarray(positives, dtype=np.float32)
    gm = np.ascontiguousarray(h[np.asarray(mix_idx)])
    ga = np.ascontiguousarray(h[np.asarray(idx_a)])
    gb = np.ascontiguousarray(h[np.asarray(idx_b)])
    ar = np.ascontiguousarray(alpha_raw, dtype=np.float32)
    br = np.ascontiguousarray(beta_raw, dtype=np.float32)
    maps = _in_maps(anchor, h, p, gm, ga, gb, ar, br)

    if os.environ.get("KERNEL_SIM", "0") == "1":
        from concourse import bass_interp

        sim = bass_interp.MultiCoreSim(nc, N_CORES)
        for c in range(N_CORES):
            for k, v in maps[c].items():
                sim.cores[c].tensor(k)[:] = v
        sim.simulate(check_with_hw=False)
        return np.asarray(
            sim.cores[0].tensor("loss")[0, 0], dtype=np.float32
        ).reshape(())

    trace = os.environ.get("BASS_KERNEL_TRACE", "0") == "1"
    res = run_bass_kernel_spmd(nc, maps, list(range(N_CORES)), trace=trace)
    global LAST_RESULTS
    LAST_RESULTS = res
    return np.asarray(res.results[0]["loss"][0, 0], dtype=np.float32).reshape(())

